# revision 9
# baseline (speedup 1.0000x reference)
"""Multi-head attention (B=2, S=2048, D=1024, H=16) on 8 TRN2 NeuronCores.

Sharding (data + tensor parallel, per the head-group hint):
  core c in 0..7 -> batch b = c // 4, head-group g = c % 4 (4 heads, 256 dims).
  Each core computes, for its batch and head group:
    QT = (x @ Wq_g + bq_g)^T          [256, 2048]   (d on partitions)
    KT likewise                       [256, 2048]
    V  = x @ Wv_g + bv_g              [2048, 256]   (S on partitions)
    per head h (4 local, Dh=64):
      ST_h = K_h @ Q_h^T              [2048k, 2048q] (scores transposed)
      E_h  = exp(ST_h / 8)            (softmax without max-subtraction; scores ~ N(0,1))
      CU_h = [V_h | 1]^T @ E_h        -> ctx^T unnormalized [64, q] + row of sums s_h[q]
      CT_h = CU_h / s_h               (ctx^T, normalized)
    OT_partial = Wo_g^T @ CT          [1024, 2048]  (out^T, partial over head groups)
  Host: out[b] = (sum_g OT_partial)^T + bo.

All heavy matmuls run in float32r (~bf16-pair precision, 4x faster than fp32
on the PE). Host passes x pre-transposed per batch so no on-chip transpose of
x is needed.
"""

import os
import numpy as np

B = 2
S = 2048
D = 1024
DL = 256          # local (per-core) d_model slice = 4 heads * 64
HL = 4            # local heads
DH = 64
QS = 512          # q tile (matmul free dim)
NQS = S // QS     # 4
KC = 128          # k chunk (psum partitions)
NKC = S // KC     # 16
DC = 128          # contraction chunk
NDC = D // DC     # 8
NCORES = 8

_RUNNER = None


def _build_program():
    import concourse.mybir as mybir
    import concourse.tile as tile
    from concourse import bacc

    F32 = mybir.dt.float32
    F32R = mybir.dt.float32r
    Ident = mybir.ActivationFunctionType.Identity
    Exp = mybir.ActivationFunctionType.Exp
    Mult = mybir.AluOpType.mult

    nc = bacc.Bacc("TRN2", target_bir_lowering=False, debug=False,
                   num_devices=NCORES)

    XT = nc.dram_tensor("XT", [D, S], F32, kind="ExternalInput").ap()
    WQ = nc.dram_tensor("WQ", [D, DL], F32, kind="ExternalInput").ap()
    WK = nc.dram_tensor("WK", [D, DL], F32, kind="ExternalInput").ap()
    WV = nc.dram_tensor("WV", [D, DL], F32, kind="ExternalInput").ap()
    WO = nc.dram_tensor("WO", [DL, D], F32, kind="ExternalInput").ap()
    # biases pre-shaped on host: BQ/BK as [128, 2] columns, BV as [1, 256]
    BQ = nc.dram_tensor("BQ", [128, 2], F32, kind="ExternalInput").ap()
    BK = nc.dram_tensor("BK", [128, 2], F32, kind="ExternalInput").ap()
    BV = nc.dram_tensor("BV", [1, DL], F32, kind="ExternalInput").ap()
    OT = nc.dram_tensor("OT", [D, S], F32, kind="ExternalOutput").ap()

    with tile.TileContext(nc) as tc:
        with (
            tc.tile_pool(name="big", bufs=1) as big,      # long-lived tensors
            tc.tile_pool(name="exp", bufs=3) as expp,     # exp(ST) tiles
            tc.tile_pool(name="outc", bufs=3) as outc,    # out-proj copies
            tc.tile_pool(name="misc", bufs=2) as misc,    # recip rows etc
            tc.tile_pool(name="psA", bufs=2, space="PSUM") as psA,
            tc.tile_pool(name="psC", bufs=4, space="PSUM") as psC,
        ):
            # ---- load inputs (gpsimd DMA casts fp32 -> fp32r on the fly) ----
            xt = big.tile([128, NDC, S], F32R, tag="xt")          # x^T
            xt_src = XT.rearrange("(c p) q -> p c q", p=128)
            for c in range(NDC):
                nc.gpsimd.dma_start(out=xt[:, c, :], in_=xt_src[:, c, :])

            wq = big.tile([128, NDC, DL], F32R, tag="wq")
            wk = big.tile([128, NDC, DL], F32R, tag="wk")
            wv = big.tile([128, NDC, DL], F32R, tag="wv")
            for w_t, W_d in ((wq, WQ), (wk, WK), (wv, WV)):
                nc.gpsimd.dma_start(
                    out=w_t, in_=W_d.rearrange("(c p) n -> p c n", p=128)
                )
            wo = big.tile([128, 2, D], F32R, tag="wo")
            nc.gpsimd.dma_start(out=wo, in_=WO.rearrange("(c p) n -> p c n", p=128))

            bqc = big.tile([128, 2], F32, tag="bqc")
            bkc = big.tile([128, 2], F32, tag="bkc")
            nc.sync.dma_start(out=bqc, in_=BQ)
            nc.sync.dma_start(out=bkc, in_=BK)
            bvr = big.tile([1, DL], F32R, tag="bvr")
            nc.gpsimd.dma_start(out=bvr, in_=BV)

            ones_f = big.tile([1, 128], F32, tag="ones_f")
            nc.vector.memset(ones_f, 1.0)
            ones_r = big.tile([1, 128], F32R, tag="ones_r")
            nc.vector.tensor_copy(out=ones_r, in_=ones_f)
            onesc_f = big.tile([128, HL, 1], F32, tag="onesc_f")
            nc.vector.memset(onesc_f, 1.0)

            # ---- projections ----
            qt = big.tile([128, 2, S], F32R, tag="qt")   # Q^T: [d(2x128), q]
            kt = big.tile([128, 2, S], F32R, tag="kt")
            for w_t, bcol, dst in ((wq, bqc, qt), (wk, bkc, kt)):
                for m in range(2):
                    for qs in range(NQS):
                        p = psA.tile([128, QS], F32, tag="mm")
                        for c in range(NDC):
                            nc.tensor.matmul(
                                p,
                                w_t[:, c, m * 128:(m + 1) * 128],
                                xt[:, c, qs * QS:(qs + 1) * QS],
                                start=(c == 0),
                                stop=(c == NDC - 1),
                            )
                        nc.scalar.activation(
                            out=dst[:, m, qs * QS:(qs + 1) * QS], in_=p,
                            func=Ident, bias=bcol[:, m:m + 1], scale=1.0,
                        )

            # V in [S, d] layout, augmented with a ones column per head
            va = big.tile([128, NKC, HL, DH + 1], F32R, tag="va")
            for sc in range(NKC):
                p = psA.tile([128, DL], F32, tag="mm")
                for c in range(NDC):
                    nc.tensor.matmul(
                        p,
                        xt[:, c, sc * 128:(sc + 1) * 128],
                        wv[:, c, :],
                        start=(c == 0),
                        stop=False,
                    )
                nc.tensor.matmul(           # += ones^T @ bv  (bias broadcast)
                    p, ones_r, bvr, start=False, stop=True,
                )
                nc.vector.tensor_copy(
                    out=va[:, sc, :, 0:DH],
                    in_=p.rearrange("p (h d) -> p h d", h=HL),
                )
                nc.gpsimd.tensor_copy(
                    out=va[:, sc, :, DH:DH + 1], in_=onesc_f
                )

            # ---- attention + out-projection ----
            ct = big.tile([128, 2, S], F32R, tag="ct")   # ctx^T: [d(2x128), q]
            for qs in range(NQS):
                for pr in range(2):          # head pair = (2pr, 2pr+1)
                    ctx = [
                        psC.tile([DH + 1, QS], F32, tag="ctx",
                                 name=f"ctx_{qs}_{pr}_{j}")
                        for j in range(2)
                    ]
                    for ki in range(NKC):
                        st = psA.tile([128, 2 * QS], F32, tag="mm")
                        for j in range(2):   # j: head-within-pair
                            nc.tensor.matmul(
                                st[:, j * QS:(j + 1) * QS],
                                kt[j * 64:(j + 1) * 64, pr, ki * KC:(ki + 1) * KC],
                                qt[j * 64:(j + 1) * 64, pr, qs * QS:(qs + 1) * QS],
                                start=True, stop=True,
                            )
                        er = expp.tile([128, 2 * QS], F32R, tag="er")
                        nc.scalar.activation(
                            out=er, in_=st, func=Exp, scale=0.125,
                        )
                        for j in range(2):
                            nc.tensor.matmul(
                                ctx[j],
                                va[:, ki, 2 * pr + j, :],
                                er[:, j * QS:(j + 1) * QS],
                                start=(ki == 0), stop=(ki == NKC - 1),
                            )
                    for j in range(2):
                        rc = misc.tile([1, QS], F32R, tag="rc")
                        # f32r recip: ~1e-5 rounding on the softmax denom, OK
                        with nc.allow_low_precision(reason="f32r softmax denom"):
                            nc.vector.reciprocal(
                                out=rc, in_=ctx[j][DH:DH + 1, :]
                            )
                        bc = psA.tile([64, QS], F32, tag="mm")
                        nc.tensor.matmul(
                            bc, ones_r[0:1, 0:64], rc, start=True, stop=True,
                        )
                        bcs = misc.tile([64, QS], F32, tag="bcs")
                        nc.vector.tensor_copy(out=bcs, in_=bc)
                        nc.vector.tensor_tensor(
                            out=ct[j * 64:(j + 1) * 64, pr, qs * QS:(qs + 1) * QS],
                            in0=ctx[j][0:DH, :],
                            in1=bcs,
                            op=Mult,
                        )

            # out-projection
            for qs in range(NQS):
                for m in range(8):
                    p = psA.tile([128, QS], F32, tag="mm")
                    for c in range(2):
                        nc.tensor.matmul(
                            p,
                            wo[:, c, m * 128:(m + 1) * 128],
                            ct[:, c, qs * QS:(qs + 1) * QS],
                            start=(c == 0), stop=(c == 1),
                        )
                    o = outc.tile([128, QS], F32, tag="o")
                    nc.vector.tensor_copy(out=o, in_=p)
                    nc.sync.dma_start(
                        out=OT[m * 128:(m + 1) * 128, qs * QS:(qs + 1) * QS],
                        in_=o,
                    )

    nc.compile()
    return nc


def _shard_inputs(x, Wq, bq, Wk, bk, Wv, bv, Wo, bo):
    x = np.asarray(x, dtype=np.float32)
    in_maps = []
    for c in range(NCORES):
        b, g = c // 4, c % 4
        sl = slice(g * DL, (g + 1) * DL)
        in_maps.append({
            "XT": np.ascontiguousarray(x[b].T),
            "WQ": np.ascontiguousarray(np.asarray(Wq, np.float32)[:, sl]),
            "WK": np.ascontiguousarray(np.asarray(Wk, np.float32)[:, sl]),
            "WV": np.ascontiguousarray(np.asarray(Wv, np.float32)[:, sl]),
            "WO": np.ascontiguousarray(np.asarray(Wo, np.float32)[sl, :]),
            # [128, 2] columns: chunk m holds bias for d-range m*128..(m+1)*128
            "BQ": np.ascontiguousarray(np.asarray(bq, np.float32)[sl].reshape(2, 128).T),
            "BK": np.ascontiguousarray(np.asarray(bk, np.float32)[sl].reshape(2, 128).T),
            "BV": np.ascontiguousarray(np.asarray(bv, np.float32)[sl].reshape(1, DL)),
        })
    return in_maps


def get_runner():
    global _RUNNER
    if _RUNNER is None:
        _RUNNER = _build_program()
    return _RUNNER


def kernel(x, Wq, bq, Wk, bk, Wv, bv, Wo, bo, **_ignored):
    from concourse.bass_utils import run_bass_kernel_spmd

    nc = get_runner()
    in_maps = _shard_inputs(x, Wq, bq, Wk, bk, Wv, bv, Wo, bo)
    res = run_bass_kernel_spmd(nc, in_maps, list(range(NCORES)))
    bo = np.asarray(bo, np.float64)
    out = np.empty((B, S, D), dtype=np.float32)
    for b in range(B):
        acc = np.zeros((D, S), dtype=np.float64)
        for g in range(4):
            acc += res.results[4 * b + g]["OT"]
        out[b] = (acc.T + bo).astype(np.float32)
    return out


# revision 19
# speedup vs baseline: 5.5347x; 5.5347x over previous
"""Multi-head attention (B=2, S=2048, D=1024, H=16) on 8 TRN2 NeuronCores.

Sharding (data + tensor parallel, per the head-group hint):
  core c in 0..7 -> batch b = c // 4, head-group g = c % 4 (4 heads, 256 dims).
  Each core computes, for its batch and head group:
    QT = (x @ Wq_g + bq_g)^T          [256, 2048]   (d on partitions)
    KT likewise                       [256, 2048]
    V  = x @ Wv_g + bv_g              [2048, 256]   (S on partitions)
    per head h (4 local, Dh=64):
      ST_h = K_h @ Q_h^T              [2048k, 2048q] (scores transposed)
      E_h  = exp(ST_h / 8)            (softmax without max-subtraction; scores ~ N(0,1))
      CU_h = [V_h | 1]^T @ E_h        -> ctx^T unnormalized [64, q] + row of sums s_h[q]
      CT_h = CU_h / s_h               (ctx^T, normalized)
    OT_partial = Wo_g^T @ CT          [1024, 2048]  (out^T, partial over head groups)
  Host: out[b] = (sum_g OT_partial)^T + bo.

All heavy matmuls run in float32r (~bf16-pair precision, 4x faster than fp32
on the PE). Host passes x pre-transposed per batch so no on-chip transpose of
x is needed.
"""

import os
import numpy as np

B = 2
S = 2048
D = 1024
DL = 256          # local (per-core) d_model slice = 4 heads * 64
HL = 4            # local heads
DH = 64
QS = 512          # q tile (matmul free dim)
NQS = S // QS     # 4
KC = 128          # k chunk (psum partitions)
NKC = S // KC     # 16
DC = 128          # contraction chunk
NDC = D // DC     # 8
NCORES = 8

_RUNNER = None


def _build_program():
    import concourse.mybir as mybir
    import concourse.tile as tile
    from concourse import bacc

    F32 = mybir.dt.float32
    F32R = mybir.dt.float32r
    Ident = mybir.ActivationFunctionType.Identity
    Exp = mybir.ActivationFunctionType.Exp
    Mult = mybir.AluOpType.mult

    nc = bacc.Bacc("TRN2", target_bir_lowering=False, debug=False,
                   num_devices=NCORES, num_swdge_queues=4)

    XT = nc.dram_tensor("XT", [D, S], F32, kind="ExternalInput").ap()
    WQ = nc.dram_tensor("WQ", [D, DL], F32, kind="ExternalInput").ap()
    WK = nc.dram_tensor("WK", [D, DL], F32, kind="ExternalInput").ap()
    WV = nc.dram_tensor("WV", [D, DL], F32, kind="ExternalInput").ap()
    WO = nc.dram_tensor("WO", [DL, D], F32, kind="ExternalInput").ap()
    # biases pre-shaped on host: BQ/BK as [128, 2] columns, BV as [1, 256]
    BQ = nc.dram_tensor("BQ", [128, 2], F32, kind="ExternalInput").ap()
    BK = nc.dram_tensor("BK", [128, 2], F32, kind="ExternalInput").ap()
    BV = nc.dram_tensor("BV", [1, DL], F32, kind="ExternalInput").ap()
    OT = nc.dram_tensor("OT", [D, S], F32, kind="ExternalOutput").ap()

    with tile.TileContext(nc) as tc:
        with (
            tc.tile_pool(name="big", bufs=1) as big,      # long-lived tensors
            tc.tile_pool(name="exp", bufs=3) as expp,     # exp(ST) tiles
            tc.tile_pool(name="outc", bufs=3) as outc,    # out-proj copies
            tc.tile_pool(name="misc", bufs=2) as misc,    # recip rows etc
            tc.tile_pool(name="psA", bufs=2, space="PSUM") as psA,
            tc.tile_pool(name="psC", bufs=3, space="PSUM") as psC,
            tc.tile_pool(name="psO", bufs=1, space="PSUM") as psO,
        ):
            # ---- load inputs (gpsimd DMA casts fp32 -> fp32r on the fly) ----
            xt = big.tile([128, NDC, S], F32R, tag="xt")          # x^T
            xt_src = XT.rearrange("(c p) q -> p c q", p=128)
            for c in range(NDC):
                nc.gpsimd.dma_start(out=xt[:, c, :], in_=xt_src[:, c, :])

            wq = big.tile([128, NDC, DL], F32R, tag="wq")
            wk = big.tile([128, NDC, DL], F32R, tag="wk")
            wv = big.tile([128, NDC, DL], F32R, tag="wv")
            for w_t, W_d in ((wq, WQ), (wk, WK), (wv, WV)):
                nc.gpsimd.dma_start(
                    out=w_t, in_=W_d.rearrange("(c p) n -> p c n", p=128)
                )

            bqc = big.tile([128, 2], F32, tag="bqc")
            bkc = big.tile([128, 2], F32, tag="bkc")
            nc.sync.dma_start(out=bqc, in_=BQ)
            nc.sync.dma_start(out=bkc, in_=BK)
            bvr = big.tile([1, DL], F32R, tag="bvr")
            nc.gpsimd.dma_start(out=bvr, in_=BV)

            # wo is needed only by the out-projection (~100us in) — load last
            wo = big.tile([128, 2, D], F32R, tag="wo")
            nc.gpsimd.dma_start(out=wo, in_=WO.rearrange("(c p) n -> p c n", p=128))

            ones_f = big.tile([1, 128], F32, tag="ones_f")
            nc.vector.memset(ones_f, 1.0)
            ones_r = big.tile([1, 128], F32R, tag="ones_r")
            nc.vector.tensor_copy(out=ones_r, in_=ones_f)
            onesc_f = big.tile([128, HL, 1], F32, tag="onesc_f")
            nc.vector.memset(onesc_f, 1.0)

            # ---- projections ----
            # Emission order QT/KT(m=0) -> V -> QT/KT(m=1): lets head-pair-0
            # attention (which needs only chunk 0 of qt/kt + va) start while
            # chunk-1 projections still run on PE.
            qt = big.tile([128, 2, S], F32R, tag="qt")   # Q^T: [d(2x128), q]
            kt = big.tile([128, 2, S], F32R, tag="kt")
            va = big.tile([128, NKC, HL, DH + 1], F32R, tag="va")

            def emit_proj_qk_one(m, qs, w_t, bcol, dst, pool):
                p = pool.tile([128, QS], F32, tag=pool.name + "p",
                              name=f"pj_{m}_{qs}_{dst.tensor.name}")
                for c in range(NDC):
                    nc.tensor.matmul(
                        p,
                        w_t[:, c, m * 128:(m + 1) * 128],
                        xt[:, c, qs * QS:(qs + 1) * QS],
                        start=(c == 0),
                        stop=(c == NDC - 1),
                    )
                nc.scalar.activation(
                    out=dst[:, m, qs * QS:(qs + 1) * QS], in_=p,
                    func=Ident, bias=bcol[:, m:m + 1], scale=1.0,
                )

            def emit_proj_qk(m, pool):
                for w_t, bcol, dst in ((wq, bqc, qt), (wk, bkc, kt)):
                    for qs in range(NQS):
                        emit_proj_qk_one(m, qs, w_t, bcol, dst, pool)

            def emit_proj_v():
                # V in [S, d] layout, augmented with a ones column per head
                for sc in range(NKC):
                    p = psA.tile([128, DL], F32, tag="mm", name=f"pv_{sc}")
                    for c in range(NDC):
                        nc.tensor.matmul(
                            p,
                            xt[:, c, sc * 128:(sc + 1) * 128],
                            wv[:, c, :],
                            start=(c == 0),
                            stop=False,
                        )
                    nc.tensor.matmul(       # += ones^T @ bv (bias broadcast)
                        p, ones_r, bvr, start=False, stop=True,
                    )
                    nc.vector.tensor_copy(
                        out=va[:, sc, :, 0:DH],
                        in_=p.rearrange("p (h d) -> p h d", h=HL),
                    )
                    nc.gpsimd.tensor_copy(
                        out=va[:, sc, :, DH:DH + 1], in_=onesc_f
                    )

            emit_proj_qk(0, psA)
            emit_proj_v()
            # chunk-1 Q/K projections are drip-fed into pair-0's attention
            # below (they use the psO pool, idle until the out-projections)

            # ---- attention, epilogue, out-projection ----
            ct = big.tile([128, 2, S], F32R, tag="ct")   # ctx^T: [d(2x128), q]

            def emit_epilogue(ctx_j, pr, qs, j):
                # normalize ctx^T by the softmax denominator (psum row 64),
                # PE-free: DVE recip -> gpsimd partition_broadcast -> DVE mult
                rc = misc.tile([1, QS], F32, tag="rc", name=f"rc_{qs}_{pr}_{j}")
                nc.vector.reciprocal(out=rc, in_=ctx_j[DH:DH + 1, :])
                bc = misc.tile([64, QS], F32, tag="bc", name=f"bc_{qs}_{pr}_{j}")
                nc.gpsimd.partition_broadcast(bc, rc, channels=64)
                nc.vector.tensor_tensor(
                    out=ct[j * 64:(j + 1) * 64, pr, qs * QS:(qs + 1) * QS],
                    in0=ctx_j[0:DH, :],
                    in1=bc,
                    op=Mult,
                )

            def emit_outproj_m(qs, m):
                p = psO.tile([128, QS], F32, tag="op", name=f"op_{qs}_{m}")
                for c in range(2):
                    nc.tensor.matmul(
                        p,
                        wo[:, c, m * 128:(m + 1) * 128],
                        ct[:, c, qs * QS:(qs + 1) * QS],
                        start=(c == 0), stop=(c == 1),
                    )
                o = outc.tile([128, QS], F32, tag="o", name=f"o_{qs}_{m}")
                nc.vector.tensor_copy(out=o, in_=p)
                nc.sync.dma_start(
                    out=OT[m * 128:(m + 1) * 128, qs * QS:(qs + 1) * QS],
                    in_=o,
                )

            # drip queue: closures emitted one-per-ki inside later segments so
            # their PE work slots into the ACT-bound attention stream
            deferred = []   # (weight, closure): weight ~ PE-cost in ki-slots
            for qs_ in range(NQS):
                for w_, b_, d_ in ((wq, bqc, qt), (wk, bkc, kt)):
                    deferred.append((4, (
                        lambda a, b, c, dd:
                        lambda: emit_proj_qk_one(1, a, b, c, dd, psO))
                        (qs_, w_, b_, d_)))
            drip_budget = 0
            for pr in range(2):              # head pair = (2pr, 2pr+1)
                for qs in range(NQS):
                    ctx = [
                        psC.tile([DH + 1, QS], F32, tag="ctx",
                                 name=f"ctx_{qs}_{pr}_{j}")
                        for j in range(2)
                    ]
                    for ki in range(NKC):
                        st = psA.tile([128, 2 * QS], F32, tag="mm",
                                      name=f"st_{qs}_{pr}_{ki}")
                        for j in range(2):   # j: head-within-pair
                            nc.tensor.matmul(
                                st[:, j * QS:(j + 1) * QS],
                                kt[j * 64:(j + 1) * 64, pr, ki * KC:(ki + 1) * KC],
                                qt[j * 64:(j + 1) * 64, pr, qs * QS:(qs + 1) * QS],
                                start=True, stop=True,
                            )
                        er = expp.tile([128, 2 * QS], F32R, tag="er")
                        nc.scalar.activation(
                            out=er, in_=st, func=Exp, scale=0.125,
                        )
                        if ki >= 2 and deferred:
                            deferred.pop(0)()
                        for j in range(2):
                            nc.tensor.matmul(
                                ctx[j],
                                va[:, ki, 2 * pr + j, :],
                                er[:, j * QS:(j + 1) * QS],
                                start=(ki == 0), stop=(ki == NKC - 1),
                            )
                    for j in range(2):
                        emit_epilogue(ctx[j], pr, qs, j)
                    if pr == 1:
                        for m in range(8):
                            deferred.append(
                                (lambda q_, m_: lambda: emit_outproj_m(q_, m_))(qs, m)
                            )
            for fn in deferred:
                fn()

    nc.compile()
    return nc


def _shard_inputs(x, Wq, bq, Wk, bk, Wv, bv, Wo, bo):
    x = np.asarray(x, dtype=np.float32)
    in_maps = []
    for c in range(NCORES):
        b, g = c // 4, c % 4
        sl = slice(g * DL, (g + 1) * DL)
        in_maps.append({
            "XT": np.ascontiguousarray(x[b].T),
            "WQ": np.ascontiguousarray(np.asarray(Wq, np.float32)[:, sl]),
            "WK": np.ascontiguousarray(np.asarray(Wk, np.float32)[:, sl]),
            "WV": np.ascontiguousarray(np.asarray(Wv, np.float32)[:, sl]),
            "WO": np.ascontiguousarray(np.asarray(Wo, np.float32)[sl, :]),
            # [128, 2] columns: chunk m holds bias for d-range m*128..(m+1)*128
            "BQ": np.ascontiguousarray(np.asarray(bq, np.float32)[sl].reshape(2, 128).T),
            "BK": np.ascontiguousarray(np.asarray(bk, np.float32)[sl].reshape(2, 128).T),
            "BV": np.ascontiguousarray(np.asarray(bv, np.float32)[sl].reshape(1, DL)),
        })
    return in_maps


def get_runner():
    global _RUNNER
    if _RUNNER is None:
        _RUNNER = _build_program()
    return _RUNNER


def kernel(x, Wq, bq, Wk, bk, Wv, bv, Wo, bo, **_ignored):
    from concourse.bass_utils import run_bass_kernel_spmd

    nc = get_runner()
    in_maps = _shard_inputs(x, Wq, bq, Wk, bk, Wv, bv, Wo, bo)
    res = run_bass_kernel_spmd(nc, in_maps, list(range(NCORES)))
    bo = np.asarray(bo, np.float64)
    out = np.empty((B, S, D), dtype=np.float32)
    for b in range(B):
        acc = np.zeros((D, S), dtype=np.float64)
        for g in range(4):
            acc += res.results[4 * b + g]["OT"]
        out[b] = (acc.T + bo).astype(np.float32)
    return out
